# revision 9
# baseline (speedup 1.0000x reference)
"""Trainium2 Bass kernel for nn_ContextBERTSelfAttention1 (banded/sparse attention).

Strategy: sequence-parallel over 8 NeuronCores. Each core owns 512 query
positions and recomputes projections over an extended window (own 512 + a
256-key halo on each side = 1024 positions) so no collectives are needed.

Per core, on device:
  - qT/kT/cqT/ckT projections in feature-major layout (weights stationary,
    host-transposed hidden states as the moving operand), v in row-major.
  - Banded scores as [128 q, 640 key] rectangles per (head, query tile):
    band slot t for row j lives at rect[j, j+t].
  - Softmax without max-subtraction (scores are O(1); masked slots get -1e9
    and underflow to exactly 0 in exp).
  - lambda gates via block-masked [768, 12] weight matrices on the PE.
  - new_probs = exp/den + lam*sigmoid(quasi) assembled with one
    scalar_tensor_tensor + one scaled copy; PV via PE transposes of the
    unscaled band, with 1/den folded into the output eviction.
  - Band extraction (rect -> [q, 513]) via a DRAM->DRAM DMA whose source
    access pattern steps the diagonal (row stride = H*640+1 elements).
"""

import os
import sys

sys.path.insert(0, "/opt/trn_rl_repo")

import numpy as np

import concourse.bass as bass
import concourse.mybir as mybir
import concourse.tile as tile
from concourse import bass_utils

# ---------------------------------------------------------------- constants
B = 1
S = 4096
E = 768
H = 12
D = 64
W = 256          # one-sided window
NCORES = 8
SS = S // NCORES         # 512 queries per core
EXT = SS + 2 * W         # 1024 extended positions (with halo)
QT = SS // 128           # 4 query tiles of 128 per core
RW = 640                 # rect width: 128 queries see 128+2*256 keys
KT = E // 128            # 6 contraction tiles
NEG = -1e9
F32 = mybir.dt.float32
F32R = mybir.dt.float32r

_cached = {}


def _split_multiwait(nc, max_waits=1):
    """This walrus build rejects instructions with >1 semaphore wait.
    Hoist extra waits onto inserted NoOps on the same engine stream."""
    import bass_rust

    cnt = 0
    f = nc.m.functions[0]
    for blk in f.blocks:
        new_insts = []
        changed = False
        for ins in blk.instructions:
            si = ins.sync_info
            if si is not None and len(si.on_wait) > max_waits:
                waits = list(si.on_wait)
                for w in waits[max_waits:]:
                    cnt += 1
                    nop = mybir.InstNoOp(name=f"waitsplit_{cnt}", ins=[], outs=[])
                    nop.engine = ins.engine
                    nop.sync_info = bass_rust.SyncInfo(on_wait=[w], on_update=[])
                    new_insts.append(nop)
                ins.sync_info = bass_rust.SyncInfo(
                    on_wait=waits[:max_waits], on_update=list(si.on_update)
                )
                changed = True
            new_insts.append(ins)
        if changed:
            blk.instructions = new_insts


def _r(ap):
    return ap.bitcast(F32R)


def _build_bass():
    nc = bass.Bass("TRN2")

    # ---------------- DRAM I/O (per-core shard) ----------------
    din = {}
    def inp(name, shape):
        din[name] = nc.dram_tensor(name, shape, F32, kind="ExternalInput")
        return din[name]

    hsT = inp("hsT", [E, EXT])          # host-transposed, halo zero-padded
    ceT = inp("ceT", [E, EXT])
    w_q = inp("w_q", [E, E])            # scale folded in on host
    w_k = inp("w_k", [E, E])
    w_v = inp("w_v", [E, E])
    w_cq = inp("w_cq", [E, E])          # scale folded in on host
    w_ck = inp("w_ck", [E, E])
    biases = inp("biases", [128, 5, KT])   # per-feature biases (q,k,v,cq,ck)
    bvrow = inp("bvrow", [1, E])           # v bias as a row (free-dim layout)
    gate_w = inp("gate_w", [E, 48])        # [Gc_q | Gq | Gc_k | Gk] blocks of 12
    gate_b = inp("gate_b", [1, 24])        # [lq bias 12 | lk bias 12]
    maskq = inp("maskq", [QT, 128, RW])    # additive mask per query tile
    onesr = inp("onesr", [1, 128])         # constant ones row
    rowm = inp("rowm", [128, QT])          # 0.0 where is_index_masked else 1.0

    out_o = nc.dram_tensor("out", [SS, E], F32, kind="ExternalOutput")
    out_p = nc.dram_tensor("new_probs", [SS, H, 2 * W + 1], F32, kind="ExternalOutput")

    with tile.TileContext(nc) as tc:
        from concourse import masks as masks_mod

        with (
            tc.tile_pool(name="resident", bufs=1) as rpool,
            tc.tile_pool(name="dram", bufs=1, space="DRAM") as dpool,
        ):
            # rect scratch in DRAM: [qt, j, h, c]
            rect_d = dpool.tile([QT, 128, H, RW], F32)

            # ---------------- resident SBUF tensors ----------------
            qT_sb = rpool.tile([128, KT, SS], F32R)      # own queries (scaled)
            kT_sb = rpool.tile([128, KT, EXT], F32R)
            cqT_sb = rpool.tile([128, KT, EXT], F32R)    # scaled context query
            v_sb = rpool.tile([128, EXT // 128, E], F32)  # row-major v
            mask_sb = rpool.tile([128, QT, RW], F32)
            bias_sb = rpool.tile([128, 5, KT], F32)
            bvrow_sb = rpool.tile([1, E], F32R)
            gatew_sb = rpool.tile([128, KT, 48], F32R)
            gateb_sb = rpool.tile([1, 24], F32R)
            rowm_sb = rpool.tile([128, QT], F32)
            ones_sb = rpool.tile([1, 128], F32R)
            ident_sb = rpool.tile([128, 128], F32)
            den_sb = rpool.tile([128, QT, H], F32)
            c1_sb = rpool.tile([128, QT, H], F32)
            recip_sb = rpool.tile([128, QT, H], F32)
            lam_sb = rpool.tile([128, QT, H], F32)
            gsig_sb = rpool.tile([128, QT, 24], F32)

            masks_mod.make_identity(nc, ident_sb[:])

            # ---------------- load constants ----------------
            def load(dst_ap, src_ap):
                nc.sync.dma_start(dst_ap, src_ap)

            load(bias_sb[:], biases.ap())
            load(ones_sb[:], onesr.ap().bitcast(F32R))
            load(bvrow_sb[:], bvrow.ap().bitcast(F32R))
            load(gateb_sb[:], gate_b.ap().bitcast(F32R))
            load(rowm_sb[:], rowm.ap())
            # gate_w [768, 48] -> [128, KT, 48]
            load(
                gatew_sb[:],
                bass.AP(gate_w, 0, [[48, 128], [128 * 48, KT], [1, 48]]).bitcast(F32R),
            )
            # maskq [QT, 128, RW] -> [128, QT, RW]
            load(
                mask_sb[:],
                bass.AP(maskq, 0, [[RW, 128], [128 * RW, QT], [1, RW]]),
            )

            with tc.tile_pool(name="projw", bufs=1) as wpool:
                hsT_sb = wpool.tile([128, KT, EXT], F32R, tag="xT")
                ceT_sb = wpool.tile([128, KT, EXT], F32R, tag="xT2")
                load(hsT_sb[:], bass.AP(hsT, 0, [[EXT, 128], [128 * EXT, KT], [1, EXT]]).bitcast(F32R))
                load(ceT_sb[:], bass.AP(ceT, 0, [[EXT, 128], [128 * EXT, KT], [1, EXT]]).bitcast(F32R))

                ckT_sb = wpool.tile([128, KT, SS], F32R, tag="ckT")

                # -------- feature-major projections: dst^T = W^T @ x^T --------
                # (tensor, weight dram, input sbuf, dst, dst cols in ext coords, bias idx)
                proj_specs = [
                    ("q", w_q, hsT_sb, qT_sb, [(W, 512)], 0),
                    ("k", w_k, hsT_sb, kT_sb, [(0, 512), (512, 512)], 1),
                    ("cq", w_cq, ceT_sb, cqT_sb, [(0, 512), (512, 512)], 3),
                    ("ck", w_ck, ceT_sb, ckT_sb, [(W, 512)], 4),
                ]
                with (
                    tc.tile_pool(name="wsb", bufs=2) as wsb_pool,
                    tc.tile_pool(name="ppsum", bufs=4, space="PSUM") as ppool,
                ):
                    for name, wdram, x_sb, dst_sb, colspecs, bidx in proj_specs:
                        w_sb = wsb_pool.tile([128, KT, E], F32R, tag="w")
                        load(
                            w_sb[:],
                            bass.AP(wdram, 0, [[E, 128], [128 * E, KT], [1, E]]).bitcast(F32R),
                        )
                        own = dst_sb is qT_sb or dst_sb is ckT_sb
                        for m in range(KT):
                            for (c0, cw) in colspecs:
                                ps = ppool.tile([128, 512], F32, tag="pp")
                                for kt in range(KT):
                                    nc.tensor.matmul(
                                        ps[:, :cw],
                                        w_sb[:, kt, 128 * m : 128 * (m + 1)],
                                        x_sb[:, kt, c0 : c0 + cw],
                                        start=(kt == 0),
                                        stop=(kt == KT - 1),
                                    )
                                d0 = c0 - W if own else c0
                                nc.scalar.activation(
                                    dst_sb[:, m, d0 : d0 + cw],
                                    ps[:, :cw],
                                    mybir.ActivationFunctionType.Identity,
                                    bias=bias_sb[:, bidx, m : m + 1],
                                )

                    # v projection (row-major), uses hsT as stationary
                    wv_sb = wsb_pool.tile([128, KT, E], F32R, tag="w")
                    load(
                        wv_sb[:],
                        bass.AP(w_v, 0, [[E, 128], [128 * E, KT], [1, E]]).bitcast(F32R),
                    )
                    for st in range(EXT // 128):
                        for (c0, cw) in [(0, 512), (512, 256)]:
                            ps = ppool.tile([128, 512], F32, tag="pp")
                            for kt in range(KT):
                                nc.tensor.matmul(
                                    ps[:, :cw],
                                    hsT_sb[:, kt, 128 * st : 128 * (st + 1)],
                                    wv_sb[:, kt, c0 : c0 + cw],
                                    start=(kt == 0),
                                    stop=False,
                                )
                            # bias via rank-1 (ones x bvrow)
                            nc.tensor.matmul(
                                ps[:, :cw],
                                ones_sb[:, :128],
                                bvrow_sb[:, c0 : c0 + cw],
                                start=False,
                                stop=True,
                            )
                            nc.vector.tensor_copy(v_sb[:, st, c0 : c0 + cw], ps[:, :cw])

                    # -------- lambda gates --------
                    # logit_lq[q,h] = cq . Wlqc + q . Wlqq + b ; lk likewise
                    for qt in range(QT):
                        gps = ppool.tile([128, 24], F32, tag="gp")
                        q0 = W + 128 * qt  # ext coords of own queries
                        for kt in range(KT):
                            nc.tensor.matmul(
                                gps[:, 0:12],
                                cqT_sb[:, kt, q0 : q0 + 128],
                                gatew_sb[:, kt, 0:12],
                                start=(kt == 0), stop=False,
                            )
                            nc.tensor.matmul(
                                gps[:, 0:12],
                                qT_sb[:, kt, 128 * qt : 128 * qt + 128],
                                gatew_sb[:, kt, 12:24],
                                start=False, stop=False,
                            )
                            nc.tensor.matmul(
                                gps[:, 12:24],
                                ckT_sb[:, kt, 128 * qt : 128 * qt + 128],
                                gatew_sb[:, kt, 24:36],
                                start=(kt == 0), stop=False,
                            )
                            nc.tensor.matmul(
                                gps[:, 12:24],
                                kT_sb[:, kt, q0 : q0 + 128],
                                gatew_sb[:, kt, 36:48],
                                start=False, stop=False,
                            )
                        nc.tensor.matmul(
                            gps[:, 0:12],
                            ones_sb[:, :128],
                            gateb_sb[:, 0:12],
                            start=False, stop=True,
                        )
                        nc.tensor.matmul(
                            gps[:, 12:24],
                            ones_sb[:, :128],
                            gateb_sb[:, 12:24],
                            start=False, stop=True,
                        )
                        nc.scalar.activation(
                            gsig_sb[:, qt, :], gps[:],
                            mybir.ActivationFunctionType.Sigmoid,
                        )
                        # lam = 1 - (lq + lk)
                        nc.vector.tensor_tensor(
                            lam_sb[:, qt, :],
                            gsig_sb[:, qt, 0:12], gsig_sb[:, qt, 12:24],
                            op=mybir.AluOpType.add,
                        )
                        nc.vector.tensor_scalar(
                            lam_sb[:, qt, :], lam_sb[:, qt, :],
                            -1.0, 1.0,
                            op0=mybir.AluOpType.mult, op1=mybir.AluOpType.add,
                        )

            # ---------------- band stage ----------------
            with (
                tc.tile_pool(name="spsum", bufs=2, space="PSUM") as spool,
                tc.tile_pool(name="tpsum", bufs=2, space="PSUM") as tpool,
                tc.tile_pool(name="band", bufs=2) as bpool,
                tc.tile_pool(name="hexp", bufs=1) as hpool,
                tc.tile_pool(name="outsb", bufs=2) as osb_pool,
            ):
                opool = tpool
                for qt in range(QT):
                    k0 = 128 * qt      # kT/cqT ext col offset of rect
                    q0 = 128 * qt      # own col offset
                    exps = []
                    sigs = []
                    # phase 1: scores + exp/sigmoid for all heads of this qt
                    for h in range(H):
                        p0 = 64 * (h % 2)
                        a = h // 2
                        s_at = spool.tile([128, RW], F32, tag="sc")
                        s_qu = spool.tile([128, RW], F32, tag="sc")
                        for (c0, cw) in [(0, 512), (512, 128)]:
                            nc.tensor.matmul(
                                s_at[:, c0 : c0 + cw],
                                qT_sb[p0 : p0 + 64, a, q0 : q0 + 128],
                                kT_sb[p0 : p0 + 64, a, k0 + c0 : k0 + c0 + cw],
                            )
                            nc.tensor.matmul(
                                s_qu[:, c0 : c0 + cw],
                                cqT_sb[p0 : p0 + 64, a, W + q0 : W + q0 + 128],
                                cqT_sb[p0 : p0 + 64, a, k0 + c0 : k0 + c0 + cw],
                            )
                        at_m = bpool.tile([128, RW], F32, tag="atm")
                        qu_m = bpool.tile([128, RW], F32, tag="qum")
                        nc.vector.tensor_tensor(
                            at_m[:], s_at[:], mask_sb[:, qt, :], op=mybir.AluOpType.add
                        )
                        nc.vector.tensor_tensor(
                            qu_m[:], s_qu[:], mask_sb[:, qt, :], op=mybir.AluOpType.add
                        )
                        e_t = hpool.tile([128, RW], F32, tag=f"exp{h}")
                        g_t = hpool.tile([128, RW], F32, tag=f"sig{h}")
                        nc.scalar.activation(
                            e_t[:], at_m[:], mybir.ActivationFunctionType.Exp,
                            accum_out=den_sb[:, qt, h : h + 1],
                        )
                        nc.scalar.activation(
                            g_t[:], qu_m[:], mybir.ActivationFunctionType.Sigmoid,
                        )
                        exps.append(e_t)
                        sigs.append(g_t)

                    # per-qt scalars: recip = rowmask/den ; c1 = lam*den
                    nc.vector.reciprocal(recip_sb[:, qt, :], den_sb[:, qt, :])
                    nc.vector.tensor_scalar(
                        recip_sb[:, qt, :], recip_sb[:, qt, :],
                        rowm_sb[:, qt : qt + 1], None,
                        op0=mybir.AluOpType.mult,
                    )
                    nc.vector.tensor_tensor(
                        c1_sb[:, qt, :], lam_sb[:, qt, :], den_sb[:, qt, :],
                        op=mybir.AluOpType.mult,
                    )

                    out_t = osb_pool.tile([128, E], F32, tag="o")
                    # phase 2: combine, emit new_probs, transpose, PV
                    for h in range(H):
                        tmp = bpool.tile([128, RW], F32, tag="tmp")
                        nc.vector.scalar_tensor_tensor(
                            tmp[:],
                            sigs[h][:], c1_sb[:, qt, h : h + 1], exps[h][:],
                            op0=mybir.AluOpType.mult, op1=mybir.AluOpType.add,
                        )
                        npb = bpool.tile([128, RW], F32, tag="npb")
                        nc.scalar.mul(npb[:], tmp[:], recip_sb[:, qt, h : h + 1])
                        nc.sync.dma_start(rect_d[qt, :, h, :], npb[:])

                        # transpose tmp -> pT (psum), evict, PV
                        pT_ps = tpool.tile([128, RW], F32, tag="pT")
                        for c in range(5):
                            nc.tensor.matmul(
                                pT_ps[:, 128 * c : 128 * (c + 1)],
                                tmp[:, 128 * c : 128 * (c + 1)],
                                ident_sb[:],
                                is_transpose=True,
                            )
                        pT_sb = bpool.tile([128, RW], F32, tag="pTs")
                        nc.vector.tensor_copy(pT_sb[:], pT_ps[:])
                        pv = opool.tile([128, D], F32, tag="pT")
                        for c in range(5):
                            nc.tensor.matmul(
                                pv[:],
                                pT_sb[:, 128 * c : 128 * (c + 1)],
                                v_sb[:, qt + c, D * h : D * (h + 1)],
                                start=(c == 0),
                                stop=(c == 4),
                            )
                        nc.scalar.mul(
                            out_t[:, D * h : D * (h + 1)], pv[:],
                            recip_sb[:, qt, h : h + 1],
                        )

                    nc.sync.dma_start(out_o.ap()[128 * qt : 128 * (qt + 1), :], out_t[:])

                    # band extraction: diagonal read from rect -> new_probs
                    rect_ap = rect_d[:]
                    diag = bass.AP(
                        rect_ap.tensor,
                        rect_ap.offset + qt * (128 * H * RW),
                        [[H * RW + 1, 128], [RW, H], [1, 2 * W + 1]],
                    )
                    nc.sync.dma_start(
                        out_p.ap()[128 * qt : 128 * (qt + 1)], diag
                    )

    _split_multiwait(nc)
    return nc


# ------------------------------------------------------------ host wrapper
def _prep_inputs(inputs):
    """Build per-core input maps (numpy only)."""
    import math

    hs = np.asarray(inputs["hidden_states"], np.float32)[0]        # [S, E]
    ce = np.asarray(inputs["context_embedded"], np.float32)[0]
    amask = np.asarray(inputs["attention_mask"]).reshape(S)
    imask = np.asarray(inputs["is_index_masked"]).reshape(S)
    scale = 1.0 / math.sqrt(D)

    w_q = np.asarray(inputs["Wq"], np.float32) * scale
    b_q = np.asarray(inputs["bq"], np.float32) * scale
    w_k = np.asarray(inputs["Wk"], np.float32)
    b_k = np.asarray(inputs["bk"], np.float32)
    w_v = np.asarray(inputs["Wv"], np.float32)
    b_v = np.asarray(inputs["bv"], np.float32)
    w_cq = np.asarray(inputs["Wcq"], np.float32) * scale
    b_cq = np.asarray(inputs["bcq"], np.float32) * scale
    w_ck = np.asarray(inputs["Wck"], np.float32)
    b_ck = np.asarray(inputs["bck"], np.float32)

    # biases in feature-major tile layout [128, 5, KT]
    biases = np.stack([b_q, b_k, b_v, b_cq, b_ck])        # [5, E]
    biases = biases.reshape(5, KT, 128).transpose(2, 0, 1).copy()

    # gate weights -> block-diagonal [E, 12] each; q gate needs *sqrt(D)
    # (device q is pre-scaled by 1/sqrt(D))
    def expand(wl):
        g = np.zeros((E, H), np.float32)
        for h in range(H):
            g[h * D : (h + 1) * D, h] = wl[:, 0]
        return g

    g_cq = expand(np.asarray(inputs["Wlqc"], np.float32))
    g_q = expand(np.asarray(inputs["Wlqq"], np.float32) / scale)
    g_ck = expand(np.asarray(inputs["Wlkc"], np.float32))
    g_k = expand(np.asarray(inputs["Wlkk"], np.float32))
    gate_w = np.concatenate([g_cq, g_q, g_ck, g_k], axis=1).copy()  # [E, 48]
    gate_b = np.concatenate(
        [
            (np.asarray(inputs["blqc"], np.float32) + np.asarray(inputs["blqq"], np.float32))
            * np.ones(H, np.float32),
            (np.asarray(inputs["blkc"], np.float32) + np.asarray(inputs["blkk"], np.float32))
            * np.ones(H, np.float32),
        ]
    ).reshape(1, 24)

    fmask = np.where(amask != 0, -10000.0, 0.0).astype(np.float32)  # [S]

    in_maps = []
    for core in range(NCORES):
        s0 = core * SS
        lo, hi = s0 - W, s0 + SS + W
        # transposed, zero-padded inputs
        hsT = np.zeros((E, EXT), np.float32)
        ceT = np.zeros((E, EXT), np.float32)
        a0, a1 = max(lo, 0), min(hi, S)
        hsT[:, a0 - lo : a1 - lo] = hs[a0:a1].T
        ceT[:, a0 - lo : a1 - lo] = ce[a0:a1].T

        # additive masks per query tile [QT, 128, RW]
        mq = np.full((QT, 128, RW), NEG, np.float32)
        for qt in range(QT):
            kstart = s0 + 128 * qt - W
            keys = kstart + np.arange(RW)
            j = np.arange(128)[:, None]
            c = np.arange(RW)[None, :]
            inband = (c - j >= 0) & (c - j <= 2 * W)
            inrange = (keys >= 0) & (keys < S)
            valid = inband & inrange[None, :]
            km = np.where(inrange, fmask[np.clip(keys, 0, S - 1)], 0.0)
            mq[qt] = np.where(valid, km[None, :], NEG)

        rowm = (~imask[s0 : s0 + SS]).astype(np.float32)
        rowm = rowm.reshape(QT, 128).T.copy()  # [128, QT]

        in_maps.append(
            {
                "hsT": hsT, "ceT": ceT,
                "w_q": w_q, "w_k": w_k, "w_v": w_v, "w_cq": w_cq, "w_ck": w_ck,
                "biases": biases, "bvrow": b_v.reshape(1, E),
                "gate_w": gate_w, "gate_b": gate_b,
                "maskq": mq, "rowm": rowm,
                "onesr": np.ones((1, 128), np.float32),
            }
        )
    return in_maps


def kernel(**inputs):
    if "nc" not in _cached:
        _cached["nc"] = _build_bass()
    nc = _cached["nc"]
    in_maps = _prep_inputs(inputs)
    trace = bool(int(os.environ.get("KERNEL_TRACE", "0")))
    if trace:
        sys.path.insert(0, os.path.dirname(os.path.abspath(__file__)))
        try:
            import axon_prof
            axon_prof.install()
        except Exception:
            pass
    res = bass_utils.run_bass_kernel_spmd(
        nc, in_maps, core_ids=list(range(NCORES)), trace=trace
    )
    _cached["last_result"] = res
    out = np.concatenate([r["out"] for r in res.results], axis=0)[None]
    new_probs = np.concatenate([r["new_probs"] for r in res.results], axis=0)[None]
    return out, new_probs
